# revision 1
# baseline (speedup 1.0000x reference)
"""Trainium2 Bass kernel for a 2-layer transformer encoder (B=8,S=1024,D=512,H=8,DK=12,DV=32,FF=2048).

Sharding: data-parallel over batch - one batch element per NeuronCore, 8 cores,
no collectives. Each core runs the full 2-layer encoder on its (S, D) slice.

v2 restructure vs baseline:
- scores: 4-way row-tiled concurrent matmuls (tile_position=(32j,0), contract
  padded 12->32 with zeros) so all 128 PE rows are active -> keeps the HAM
  clock-gate warm (baseline ran attention at 1.2 GHz half-clock).
- ctx: 4-way col-tiled concurrent matmuls (tile_position=(0,32j)), softmax
  denominators via separate 4-way col-tiled ones-vector matmuls.
- out-proj: 4-way row-tiled accumulation (contract 32 per head).
- q-half software pipelining: FFN/out-proj of token-half h overlaps the exp
  (ACT) work of token-half h+1.
- LN rstd via exp(-0.5*ln(var)) so ACT stays in the natural_log_exp table set
  (no table switches between LN and attention exp).
- LN normalize on DVE via tensor_scalar with two per-partition scalars.

Self-contained: hardcodes all shapes; host side only reshapes/casts/shards.
"""

import sys

sys.path.insert(0, "/opt/trn_rl_repo")

import numpy as np
import ml_dtypes

import concourse.bass as bass
import concourse.tile as tile
from concourse import bacc, mybir
from concourse.masks import make_identity

F32 = mybir.dt.float32
BF16 = mybir.dt.bfloat16
I32 = mybir.dt.int32

L = 2
S = 1024
D = 512
H = 8
DK = 12
DV = 32
FF = 2048
SM = S // 128   # 8 token tiles
DC = D // 128   # 4 D-chunks
FC = FF // 128  # 16 FF-chunks
SCALE = float(1.0 / np.sqrt(np.float32(DK)))
NCORES = 8

AF = mybir.ActivationFunctionType
ALU = mybir.AluOpType


def build_module(with_mask=False):
    nc = bacc.Bacc("TRN2", target_bir_lowering=False, debug=False, num_devices=NCORES)

    x_in = nc.dram_tensor("x", [S, D], F32, kind="ExternalInput")
    wq_d = nc.dram_tensor("wq", [L, DC, 128, 256], BF16, kind="ExternalInput")
    wk_d = nc.dram_tensor("wk", [L, DC, 128, 256], BF16, kind="ExternalInput")
    wv_d = nc.dram_tensor("wv", [L, DC, 128, 256], BF16, kind="ExternalInput")
    wx_d = nc.dram_tensor("wx", [L, 128, 2, D], BF16, kind="ExternalInput")
    w1_d = nc.dram_tensor("w1", [L, DC, 128, FF], BF16, kind="ExternalInput")
    w2_d = nc.dram_tensor("w2", [L, FC, 128, D], BF16, kind="ExternalInput")
    mask_d = None
    if with_mask:
        mask_d = nc.dram_tensor("maskf", [S], F32, kind="ExternalInput")
    out_d = nc.dram_tensor("out", [S, D], F32, kind="ExternalOutput")
    out_ap = out_d.rearrange("(m p) d -> p m d", p=128)

    with tile.TileContext(nc) as tc:
        with (
            tc.tile_pool(name="const", bufs=1) as const,
            tc.tile_pool(name="wts", bufs=2) as wts,
            tc.tile_pool(name="wbig", bufs=2) as wbig,
            tc.tile_pool(name="acts", bufs=1) as acts,
            tc.tile_pool(name="nx1p", bufs=3) as nx1p,
            tc.tile_pool(name="nx2p", bufs=3) as nx2p,
            tc.tile_pool(name="trs", bufs=2) as trs,
            tc.tile_pool(name="pt", bufs=2) as ptp,
            tc.tile_pool(name="kvp", bufs=2) as kvp,
            tc.tile_pool(name="hT", bufs=1) as htp,
            tc.tile_pool(name="small", bufs=2) as small,
            tc.tile_pool(name="mult", bufs=2) as multp,
            tc.tile_pool(name="norm1", bufs=1) as normp,
            tc.tile_pool(name="ps_sp", bufs=1, space="PSUM") as ps_sp,
            tc.tile_pool(name="ps_cq", bufs=1, space="PSUM") as ps_cq,
            tc.tile_pool(name="ps", bufs=2, space="PSUM") as psb,
        ):
            ident = const.tile([128, 128], F32)
            make_identity(nc, ident)
            ones = const.tile([128, 1], BF16)
            nc.vector.memset(ones[:], 1.0)

            # residual stream, token-major: x[:, m, :] is tokens 128m..128m+127
            x = acts.tile([128, SM, D], F32, tag="x")
            xsrc = x_in.rearrange("(m p) d -> p m d", p=128)
            for m in range(SM):
                nc.sync.dma_start(out=x[:, m, :], in_=xsrc[:, m, :])

            mask_sb = None
            if with_mask:
                mask_sb = const.tile([128, SM], F32)
                nc.sync.dma_start(
                    out=mask_sb[:], in_=mask_d.rearrange("(m p) -> p m", p=128)
                )

            # per-layer weights (bufs=2 rotates across layers)
            W = []
            for l in range(L):
                wq = wts.tile([128, DC, 256], BF16, tag="wq")
                wk = wts.tile([128, DC, 256], BF16, tag="wk")
                wv = wts.tile([128, DC, 256], BF16, tag="wv")
                wx = wts.tile([128, 2, D], BF16, tag="wx")
                w1 = wbig.tile([128, DC, FF], BF16, tag="w1")
                w2 = wbig.tile([128, FC, D], BF16, tag="w2")
                nc.sync.dma_start(out=wq[:], in_=wq_d[l].rearrange("c p n -> p c n"))
                nc.sync.dma_start(out=wk[:], in_=wk_d[l].rearrange("c p n -> p c n"))
                nc.sync.dma_start(out=wv[:], in_=wv_d[l].rearrange("c p n -> p c n"))
                nc.sync.dma_start(out=wx[:], in_=wx_d[l])
                nc.sync.dma_start(out=w1[:], in_=w1_d[l].rearrange("c p n -> p c n"))
                nc.sync.dma_start(out=w2[:], in_=w2_d[l].rearrange("c p n -> p c n"))
                W.append((wq, wk, wv, wx, w1, w2))

            # qt/kt/v are double-buffered per layer so next-layer
            # projections never clobber tensors the current layer's
            # attention still reads.
            ctxT = acts.tile([128, 2, S], BF16, tag="ctxT")

            def alloc_kv():
                return dict(
                    qt=kvp.tile([128, 2, S], BF16, tag="qt", name="qt"),
                    kt=kvp.tile([128, 2, S], BF16, tag="kt", name="kt"),
                    v=kvp.tile([128, SM, 256], BF16, tag="v", name="v"),
                )

            def emit_ln(xt, m, nx_tiles, pool):
                """LN stats + normalize for token tile m (DVE + tiny ACT)."""
                st = small.tile([128, 6], F32, tag="bnst", name="bnst")
                mv = small.tile([128, 2], F32, tag="bnmv", name="bnmv")
                nc.vector.bn_stats(out=st[:], in_=xt[:, m, :])
                nc.vector.bn_aggr(out=mv[:], in_=st[:])
                # rstd = (var*D/(D-1))^-0.5 entirely on DVE (quake seed +
                # one Newton step, ~0.2% max err) - keeps ACT exp-only so the
                # activation table never switches.
                ti = small.tile([128, 1], I32, tag="ti", name="ti")
                rstd = small.tile([128, 1], F32, tag="rstd", name="rstd")
                u = small.tile([128, 1], F32, tag="u", name="u")
                nc.vector.tensor_scalar(
                    out=ti[:], in0=mv[:, 1:2].bitcast(I32), scalar1=1, scalar2=None,
                    op0=ALU.logical_shift_right,
                )
                nc.vector.tensor_scalar(
                    out=rstd[:].bitcast(I32), in0=ti[:], scalar1=-1,
                    scalar2=0x5F3759DF, op0=ALU.mult, op1=ALU.add,
                )
                nc.vector.tensor_tensor(out=u[:], in0=rstd[:], in1=rstd[:], op=ALU.mult)
                nc.vector.tensor_tensor(out=u[:], in0=u[:], in1=mv[:, 1:2], op=ALU.mult)
                nc.vector.tensor_scalar(
                    out=u[:], in0=u[:], scalar1=-0.5 * float(D) / (D - 1), scalar2=1.5,
                    op0=ALU.mult, op1=ALU.add,
                )
                nc.vector.tensor_tensor(out=rstd[:], in0=rstd[:], in1=u[:], op=ALU.mult)
                nmr = small.tile([128, 1], F32, tag="nmr", name="nmr")
                nc.vector.scalar_tensor_tensor(
                    out=nmr[:], in0=mv[:, 0:1], scalar=-1.0, in1=rstd[:],
                    op0=ALU.mult, op1=ALU.mult,
                )
                nx = pool.tile([128, D], F32, tag="nx", name="nx")
                nc.vector.tensor_scalar(
                    out=nx[:], in0=xt[:, m, :], scalar1=rstd[:], scalar2=nmr[:],
                    op0=ALU.mult, op1=ALU.add,
                )
                nx_tiles[m] = nx

            def emit_transposes(nx_tiles, nT, ms, moff=0):
                """PE transposes of normalized tiles ms -> nT[:, :, 128(m-moff)..] bf16."""
                for m in ms:
                    nx = nx_tiles.pop(m)
                    tp = psb.tile([128, 512], F32, tag="ps", name="tp")
                    for c in range(DC):
                        nc.tensor.transpose(
                            tp[:, 128 * c:128 * (c + 1)], nx[:, 128 * c:128 * (c + 1)],
                            ident[:],
                        )
                    lm = m - moff
                    nc.vector.tensor_copy(
                        out=nT[:, :, 128 * lm:128 * (lm + 1)],
                        in_=tp[:].rearrange("p (c t) -> p c t", c=DC),
                    )

            def emit_qk_proj(kv, key, w, nT, quad, th):
                dst = kv[key]
                pp = psb.tile([128, 512], F32, tag="ps", name="pp")
                for c in range(DC):
                    nc.tensor.matmul(
                        pp[:], w[:, c, 128 * quad:128 * (quad + 1)],
                        nT[:, c, 512 * th:512 * (th + 1)],
                        start=(c == 0), stop=(c == DC - 1),
                    )
                nc.scalar.copy(out=dst[:, quad, 512 * th:512 * (th + 1)], in_=pp[:])

            def emit_v_proj_m(kv, nT, wv, m):
                v = kv["v"]
                pp = psb.tile([128, 512], F32, tag="ps", name="pp")
                for c in range(DC):
                    nc.tensor.matmul(
                        pp[:, 0:256],
                        nT[:, c, 128 * m:128 * (m + 1)],
                        wv[:, c, :],
                        start=(c == 0), stop=(c == DC - 1),
                    )
                nc.scalar.copy(out=v[:, m, :], in_=pp[:, 0:256])

            def emit_kvq_half(kv, nT, wq, wk, wv, th):
                cl = []
                for quad in range(2):
                    cl.append((emit_qk_proj, (kv, "kt", wk, nT, quad, th)))
                for m in range(4 * th, 4 * th + 4):
                    cl.append((emit_v_proj_m, (kv, nT, wv, m)))
                for quad in range(2):
                    cl.append((emit_qk_proj, (kv, "qt", wq, nT, quad, th)))
                return cl

            def emit_scores_exp(kv, quad, mk, qh):
                """4 row-tiled concurrent score MMs + one exp -> pt tile."""
                kt, qt = kv["kt"], kv["qt"]
                sp = ps_sp.tile([128, 4, 512], F32, tag="sp", name="sp")
                for j in range(4):
                    nc.tensor.matmul(
                        sp[:, j, :],
                        kt[32 * j:32 * j + 32, quad, 128 * mk:128 * (mk + 1)],
                        qt[32 * j:32 * j + 32, quad, 512 * qh:512 * (qh + 1)],
                        start=True, stop=True,
                        tile_position=(32 * j, 0),
                    )
                pt = ptp.tile([128, 4, 512], BF16, tag="pt", name="pt")
                nc.scalar.activation(out=pt[:], in_=sp[:], func=AF.Exp, scale=SCALE)
                if with_mask:
                    nc.vector.tensor_scalar_mul(
                        out=pt[:], in0=pt[:], scalar1=mask_sb[:, mk:mk + 1]
                    )
                return pt

            def emit_ctx(kv, quad, mk, pt, cq):
                """4 col-tiled ctx MMs + 4 col-tiled denominator MMs."""
                v = kv["v"]
                for j in range(4):
                    h = 4 * quad + j
                    nc.tensor.matmul(
                        cq[32 * j:32 * j + 32, 0, :],
                        v[:, mk, 32 * h:32 * h + 32],
                        pt[:, j, :],
                        start=(mk == 0), stop=(mk == SM - 1),
                        tile_position=(0, 32 * j),
                    )
                for j in range(4):
                    nc.tensor.matmul(
                        cq[32 * j:32 * j + 1, 1, :],
                        ones[:],
                        pt[:, j, :],
                        start=(mk == 0), stop=(mk == SM - 1),
                        tile_position=(0, 32 * j),
                    )

            def emit_ctx_norm(quad, qh, cq):
                # Evacuate cq to SBUF with two fast copies so the single-buf
                # PSUM tile frees immediately; the bcast/DMA-assembly chain
                # then runs off the PE-critical path.
                cqc = multp.tile([128, 512], F32, tag="cqc", name="cqc")
                nc.vector.tensor_copy(out=cqc[:], in_=cq[:, 0, :])
                den97 = normp.tile([97, 512], F32, tag="den97", name="den97")
                nc.vector.tensor_copy(out=den97[:], in_=cq[0:97, 1, :])
                # partition_broadcast needs base-0 in/out; DMA does the
                # partition staging and band assembly (DMA engines are idle).
                mult = normp.tile([128, 512], F32, tag="mult", name="mult")
                for j in range(4):
                    dj = normp.tile([1, 512], F32, tag="denj", name="denj")
                    nc.sync.dma_start(out=dj[0:1, :], in_=den97[32 * j:32 * j + 1, :])
                    bj = multp.tile([32, 512], F32, tag="bj", name="bj")
                    nc.gpsimd.partition_broadcast(bj[:], dj[0:1, :])
                    nc.sync.dma_start(out=mult[32 * j:32 * j + 32, :], in_=bj[:])
                rec = normp.tile([128, 512], F32, tag="rec", name="rec")
                nc.vector.reciprocal_approx_fast(out=rec[:], in_=mult[:])
                nc.vector.scalar_tensor_tensor(
                    out=ctxT[:, quad, 512 * qh:512 * (qh + 1)],
                    in0=cqc[:], scalar=1.0, in1=rec[:],
                    op0=ALU.mult, op1=ALU.mult,
                )

            def emit_outproj_ln2(m, wx, nx_tiles):
                ap_ = psb.tile([128, 512], F32, tag="ps", name="ap_")
                for quad in range(2):
                    nc.tensor.matmul(
                        ap_[:],
                        ctxT[:, quad, 128 * m:128 * (m + 1)],
                        wx[:, quad, :],
                        start=(quad == 0), stop=(quad == 1),
                    )
                nc.vector.tensor_add(out=x[:, m, :], in0=ap_[:], in1=x[:, m, :])
                emit_ln(x, m, nx_tiles, nx2p)

            def emit_ffn1(ff, w1, n2T, hT):
                hp = psb.tile([128, 512], F32, tag="ps", name="hp")
                for c in range(DC):
                    nc.tensor.matmul(
                        hp[:], w1[:, c, 128 * ff:128 * (ff + 1)],
                        n2T[:, c, :],
                        start=(c == 0), stop=(c == DC - 1),
                    )
                nc.vector.tensor_scalar_max(
                    out=hT[:, ff, :], in0=hp[:], scalar1=0.0
                )

            def emit_ffn2(m, qh, w2, hT, l, nTn):
                lm = m - 4 * qh
                yp = psb.tile([128, 512], F32, tag="ps", name="yp")
                for ff in range(FC):
                    nc.tensor.matmul(
                        yp[:], hT[:, ff, 128 * lm:128 * (lm + 1)], w2[:, ff, :],
                        start=(ff == 0), stop=(ff == FC - 1),
                    )
                nc.vector.tensor_add(out=x[:, m, :], in0=yp[:], in1=x[:, m, :])
                if l < L - 1:
                    tmp = {}
                    emit_ln(x, m, tmp, nx1p)
                    emit_transposes(tmp, nTn, [m])
                else:
                    nc.sync.dma_start(out=out_ap[:, m, :], in_=x[:, m, :])

            # ---------------- program ----------------
            # Rolling pipeline: every (quad, mk) step of an attention loop
            # drips closures of the previous token-half's out-proj/FFN (and
            # the projections they enable), so PE work always overlaps the
            # ACT exp stream and the HAM clock-gate stays warm.
            nTs = {}

            KV = {}

            def make_tail(l, qh):
                wq_, wk_, wv_, wx_, w1_, w2_ = W[l]
                cl = []
                n2T = trs.tile([128, DC, 512], BF16, tag="n2T", name="n2T")
                hTq = htp.tile([128, FC, 512], BF16, tag="hT", name="hT")
                nx2 = {}
                for m in range(4 * qh, 4 * qh + 4):
                    cl.append((emit_outproj_ln2, (m, wx_, nx2)))
                    cl.append((emit_transposes, (nx2, n2T, [m], 4 * qh)))
                for ff in range(FC):
                    cl.append((emit_ffn1, (ff, w1_, n2T, hTq)))
                if l < L - 1:
                    nTn = nTs[l + 1]
                    for m in range(4 * qh, 4 * qh + 4):
                        cl.append((emit_ffn2, (m, qh, w2_, hTq, l, nTn)))
                    wqn, wkn, wvn = W[l + 1][0], W[l + 1][1], W[l + 1][2]
                    if qh == 0:
                        KV[l + 1] = alloc_kv()
                    cl += emit_kvq_half(KV[l + 1], nTn, wqn, wkn, wvn, qh)
                else:
                    for m in range(4 * qh, 4 * qh + 4):
                        cl.append((emit_ffn2, (m, qh, w2_, hTq, l, None)))
                return cl

            def attn_loop(l, qh, pending):
                # For qh==0 the queue ends with this layer's token-half-1
                # K/V/Q projections, which the mk>=4 scores read: drain the
                # whole queue before the first mk>=4 group (program order).
                kv = KV[l]
                done = 0
                npend = len(pending)
                slots = 16
                prev = None
                for quad in range(2):
                    cq = ps_cq.tile([128, 2, 512], F32, tag="cq", name="cq")
                    for mk in range(SM):
                        if qh == 0 and mk == 4:
                            want = npend
                        else:
                            it = quad * SM + mk + 1
                            want = (npend * max(0, it - 2)) // (slots - 2)
                        while done < want:
                            fn, args = pending[done]
                            fn(*args)
                            done += 1
                        pt = emit_scores_exp(kv, quad, mk, qh)
                        if prev is not None:
                            pq, pm, ppt, pcq = prev
                            emit_ctx(kv, pq, pm, ppt, pcq)
                            if pm == SM - 1:
                                emit_ctx_norm(pq, qh, pcq)
                        prev = (quad, mk, pt, cq)
                pq, pm, ppt, pcq = prev
                emit_ctx(kv, pq, pm, ppt, pcq)
                emit_ctx_norm(pq, qh, pcq)
                while done < npend:
                    fn, args = pending[done]
                    fn(*args)
                    done += 1

            # boot: layer-0 LN1 + transposes + token-half-0 projections run
            # serially; half-1 projections become the first drip queue.
            for l in range(L):
                nTs[l] = trs.tile([128, DC, S], BF16, tag="nT", name="nT")
            wq0, wk0, wv0 = W[0][0], W[0][1], W[0][2]
            KV[0] = alloc_kv()
            nx_boot = {}
            for m in range(SM):
                emit_ln(x, m, nx_boot, nx1p)
                emit_transposes(nx_boot, nTs[0], [m])
            for fn, args in emit_kvq_half(KV[0], nTs[0], wq0, wk0, wv0, 0):
                fn(*args)

            pending = emit_kvq_half(KV[0], nTs[0], wq0, wk0, wv0, 1)
            for l in range(L):
                attn_loop(l, 0, pending)
                pending = make_tail(l, 0)
                attn_loop(l, 1, pending)
                pending = make_tail(l, 1)
            # final tail (layer L-1 half 1) runs serially; its FFN2 closures
            # stream the output DMAs per token tile.
            for fn, args in pending:
                fn(*args)

    nc.compile()
    return nc


_CACHE = {}


def _get_module(with_mask):
    key = (with_mask,)
    if key not in _CACHE:
        _CACHE[key] = build_module(with_mask=with_mask)
    return _CACHE[key]


def _prep_weights(Wq, Wk, Wv, Wx, W1, W2):
    bf = ml_dtypes.bfloat16

    # Q/K: pad head columns from 12 to 32 (head h=4q+j at col 128q+32j)
    def pad_qk(w):  # [L, 512, 96] -> [L, DC, 128, 256]
        out = np.zeros((L, D, 256), np.float32)
        for h in range(H):
            q, j = divmod(h, 4)
            out[:, :, 128 * q + 32 * j:128 * q + 32 * j + DK] = (
                w[:, :, DK * h:DK * (h + 1)]
            )
        return np.ascontiguousarray(out.reshape(L, DC, 128, 256)).astype(bf)

    wq = pad_qk(np.asarray(Wq))
    wk = pad_qk(np.asarray(Wk))
    wv = np.ascontiguousarray(np.asarray(Wv).reshape(L, DC, 128, 256)).astype(bf)
    # Wx rows (h=4q+j, dd) -> [32j+dd, quad, :]
    wx = np.ascontiguousarray(
        np.asarray(Wx).reshape(L, 2, 4, 32, D).transpose(0, 2, 3, 1, 4)
        .reshape(L, 128, 2, D)
    ).astype(bf)
    w1 = np.ascontiguousarray(np.asarray(W1).reshape(L, DC, 128, FF)).astype(bf)
    w2 = np.ascontiguousarray(np.asarray(W2).reshape(L, FC, 128, D)).astype(bf)
    return dict(wq=wq, wk=wk, wv=wv, wx=wx, w1=w1, w2=w2)


def kernel(inputs, mask, Wq, bq, Wk, bk, Wv, bv, Wx, bx, W1, b1, W2, b2, gamma, beta):
    inputs = np.asarray(inputs, np.float32)
    mask = np.asarray(mask)
    for nm, b in (("bq", bq), ("bk", bk), ("bv", bv), ("bx", bx), ("b1", b1), ("b2", b2)):
        assert not np.any(np.asarray(b)), f"nonzero bias {nm} not supported"
    assert np.all(np.asarray(gamma) == 1.0) and not np.any(np.asarray(beta)), (
        "non-identity layernorm affine not supported"
    )

    with_mask = bool(np.any(np.asarray(mask) == 0))
    nc = _get_module(with_mask)
    wmap = _prep_weights(
        np.asarray(Wq, np.float32), np.asarray(Wk, np.float32),
        np.asarray(Wv, np.float32), np.asarray(Wx, np.float32),
        np.asarray(W1, np.float32), np.asarray(W2, np.float32),
    )

    in_maps = []
    for b in range(NCORES):
        m = dict(wmap)
        m["x"] = np.ascontiguousarray(inputs[b])
        if with_mask:
            m["maskf"] = np.ascontiguousarray((mask[b, 0] != 0).astype(np.float32))
        in_maps.append(m)

    import os
    from concourse.bass_utils import run_bass_kernel_spmd

    kw = {}
    tdir = os.environ.get("BASS_KERNEL_TRACE_DIR")
    if tdir:
        kw = dict(trace=True, tmpdir=tdir)
    res = run_bass_kernel_spmd(nc, in_maps, core_ids=list(range(NCORES)), **kw)
    global LAST_EXEC_NS
    LAST_EXEC_NS = res.exec_time_ns
    out = np.stack([res.results[i]["out"] for i in range(NCORES)], axis=0)
    return out.astype(np.float32)


LAST_EXEC_NS = None

